# revision 1
# baseline (speedup 1.0000x reference)
"""Heavy-hitter (H2O) LlamaAttention, sharded over 8 trn2 NeuronCores.

Sharding: tensor-parallel over heads — each of the 8 cores owns 4 of the 32
heads (q/k/v column-parallel, o_proj row-parallel); host sums the 8 partial
o_proj outputs.

The heavy-hitter scan is reformulated exactly as a death-time process
(verified against the reference scan): the alive set always has 102 columns,
exactly one column is evicted per step (the argmin of accumulated prob mass),
and the final mask depends only on each column's death step d_c via
  keep[t,c] = (c <= t) & (t < D_c),  D_c = inf for c<4 else max(d_c, c+103).
Evictions of the newest column (the overwhelmingly common case) never affect
the mask, so only the rare incumbent-replacement "events" (~3/head) need
sequential resolution.  The O(S^2) prefix-sum matrix C and all row minima are
computed on-device; the host only walks the ~3 events per head using tiny
[4,1024] reductions fetched per pass.
"""
import os
os.environ.setdefault("NEURON_CC_FLAGS", "--auto-cast=none")

import time
import numpy as np
import jax
import jax.numpy as jnp

S, D, H = 1024, 4096, 32
HD = D // H           # 128
HB = 102              # heavy budget
RB = 102              # recent budget
NCORES = 8
HPC = H // NCORES     # 4 heads per core
EPC = HPC * HD        # 512 dims per core
NEG = float(np.finfo(np.float32).min)


def _rotary_tables():
    inv_freq = 1.0 / (10000.0 ** (np.arange(0, HD, 2, dtype=np.float32) / HD))
    t = np.arange(S, dtype=np.float32)
    freqs = np.outer(t, inv_freq)
    emb = np.concatenate([freqs, freqs], -1)
    return np.cos(emb).astype(np.float32), np.sin(emb).astype(np.float32)


def _rot_half(x):
    x1, x2 = jnp.split(x, 2, axis=-1)
    return jnp.concatenate([-x2, x1], axis=-1)


def _qk_scores(x, mask, Wq_s, Wk_s, cos, sin):
    # x [S,D]; Wq_s/Wk_s [EPC, D] (rows for this core's heads); mask [S,S]
    q = jnp.einsum('sd,ed->se', x, Wq_s).reshape(S, HPC, HD).transpose(1, 0, 2)
    k = jnp.einsum('sd,ed->se', x, Wk_s).reshape(S, HPC, HD).transpose(1, 0, 2)
    c = cos[None]; s = sin[None]
    q = q * c + _rot_half(q) * s
    k = k * c + _rot_half(k) * s
    attn = jnp.einsum('hqd,hkd->hqk', q, k) / jnp.sqrt(jnp.float32(HD))
    attn = jnp.maximum(attn + mask[None], NEG)
    return attn


def _phase1(x, mask, Wq_s, Wk_s, cos, sin, Ltri):
    attn = _qk_scores(x, mask, Wq_s, Wk_s, cos, sin)
    probs = jax.nn.softmax(attn, axis=-1)          # [HPC,S,S]
    # C_excl[h,t,c] = sum_{s<t} probs[h,s,c]
    C = jnp.einsum('ts,hsc->htc', Ltri, probs)
    nb = jnp.diagonal(C, offset=-1, axis1=1, axis2=2)  # [HPC, S-1]; nb[:,i] = C[:,i+1,i]
    return C, nb


def _min_pass(C, pen):
    # C [HPC,S,S], pen [HPC,S] over columns -> per-row min/argmin over penalized cols
    V = C + pen[:, None, :]
    return jnp.min(V, axis=-1), jnp.argmin(V, axis=-1).astype(jnp.int32)


def _phase2(x, mask, Wq_s, Wk_s, Wv_s, Wo_s, cos, sin, Dv):
    attn = _qk_scores(x, mask, Wq_s, Wk_s, cos, sin)
    rows = jnp.arange(S)[:, None]
    cols = jnp.arange(S)[None, :]
    keep = (cols <= rows)[None] & (rows[None] < Dv[:, None, :])  # [HPC,S,S]
    attn = jnp.where(keep, attn, NEG)
    attn = jax.nn.softmax(attn.astype(jnp.float32), axis=-1)
    v = jnp.einsum('sd,ed->se', x, Wv_s).reshape(S, HPC, HD).transpose(1, 0, 2)
    out = jnp.einsum('hqk,hkd->hqd', attn, v)               # [HPC,S,HD]
    out = out.transpose(1, 0, 2).reshape(S, EPC)
    return jnp.einsum('se,de->sd', out, Wo_s)               # [S,D] partial


def kernel(hidden_states, attention_mask, Wq, Wk, Wv, Wo):
    t0 = time.time()
    x = np.asarray(hidden_states, dtype=np.float32)[0]          # [S,D]
    am = np.asarray(attention_mask, dtype=np.float32)[0, 0]     # [S,S]
    cos, sin = _rotary_tables()
    Ltri = np.tril(np.ones((S, S), dtype=np.float32), k=-1)

    # per-core weight shards
    Wq_s = np.stack([Wq[i * EPC:(i + 1) * EPC, :] for i in range(NCORES)])
    Wk_s = np.stack([Wk[i * EPC:(i + 1) * EPC, :] for i in range(NCORES)])
    Wv_s = np.stack([Wv[i * EPC:(i + 1) * EPC, :] for i in range(NCORES)])
    Wo_s = np.stack([Wo[:, i * EPC:(i + 1) * EPC] for i in range(NCORES)])  # [4096,512]
    xB = np.broadcast_to(x, (NCORES,) + x.shape)
    amB = np.broadcast_to(am, (NCORES,) + am.shape)
    cosB = np.broadcast_to(cos, (NCORES,) + cos.shape)
    sinB = np.broadcast_to(sin, (NCORES,) + sin.shape)
    LtriB = np.broadcast_to(Ltri, (NCORES,) + Ltri.shape)

    p1 = jax.pmap(_phase1)
    pmin = jax.pmap(_min_pass)
    p2 = jax.pmap(_phase2)

    C, nb = p1(xB, amB, Wq_s, Wk_s, cosB, sinB, LtriB)
    nb_h = np.asarray(nb)                                       # [8,HPC,S-1]

    # host event loop: resolve incumbent replacements per (core, head)
    BIGP = np.float32(1e30)
    pen = np.full((NCORES, HPC, S), BIGP, dtype=np.float32)
    pen[:, :, :HB - 1] = 0.0                                    # J = {0..100}
    cursor = np.full((NCORES, HPC), HB, dtype=np.int64)         # first step = 102
    deaths = [[dict() for _ in range(HPC)] for _ in range(NCORES)]
    for _pass in range(40):
        pending = [(i, h) for i in range(NCORES) for h in range(HPC)
                   if cursor[i, h] < S]
        if not pending:
            break
        mins, amins = pmin(C, jnp.asarray(pen))
        mins = np.asarray(mins); amins = np.asarray(amins)
        for i, h in pending:
            t = int(cursor[i, h])
            mJ = mins[i, h, t:]                 # min over J at rows t..S-1
            aJ = amins[i, h, t:]
            nbacc = nb_h[i, h, t - 1:]          # newborn col tt-1 acc at row tt
            surv = nbacc > mJ                   # event: newborn beats incumbent min
            idx = np.flatnonzero(surv)
            if idx.size == 0:
                cursor[i, h] = S
                continue
            te = t + int(idx[0])                # event step
            victim = int(aJ[idx[0]])
            deaths[i][h][victim] = te
            pen[i, h, victim] = BIGP
            pen[i, h, te - 1] = 0.0             # newborn joins J
            cursor[i, h] = te + 1

    # build D_c per (core, head)
    Dv = np.empty((NCORES, HPC, S), dtype=np.float32)
    base = np.minimum(np.arange(S, dtype=np.float32) + (RB + 1), np.float32(2 * S))
    for i in range(NCORES):
        for h in range(HPC):
            d = base.copy()
            for c, t in deaths[i][h].items():
                d[c] = max(float(t), base[c])
            d[:4] = 2 * S                        # forced cols never masked
            # survivors: columns never recorded dead that are current incumbents
            alive = pen[i, h] == 0.0
            d[alive] = 2 * S
            Dv[i, h] = d

    part = p2(xB, amB, Wq_s, Wk_s, Wv_s, Wo_s, cosB, sinB, jnp.asarray(Dv))
    out = np.asarray(part).sum(axis=0, dtype=np.float32)[None]  # [1,S,D]
    kernel.elapsed_ns = int((time.time() - t0) * 1e9)
    return out.astype(np.float32)

